# revision 11
# baseline (speedup 1.0000x reference)
"""Multi-head GAT layer (4 heads, cat) on 8 TRN2 NeuronCores.

Strategy (edge-parallel by dst, per-core full z table):
  - dst = repeat(arange(N), 16)  (verified): edges pre-grouped by dst node.
  - Core c owns dst nodes [6250c, 6250c+6250) and their 100k edges.
  - Phase 1 (per core): z_aug = h @ [W | u_src | u_dst] -> HBM table
    [50176 rows x 384 bf16] (row = 256 z + 4 es + 4 ed + pad to 768B).
  - Phase 2: per group of 128 dst nodes, gather the 2048 source rows with
    two dma_gather calls (int16 idx => two table windows [0,32768) and
    [17408,50176), host-paired "pair-lane" layout so both calls are exactly
    1024 indices with zero padding), then:
      logits e = es[src] + ed[dst] (ed via pattern matmuls), leaky-relu, exp
      weighted segment sums + denominators via PE matmuls with data-driven
      one-hot masks (host-precomputed EQ tensors), division folded into the
      PSUM evacuation scale.
  - Output rows are in host-chosen (paired) order; host inverts at the end.
"""

import numpy as np
import ml_dtypes

N = 50000
DEG = 16
IN_DIM = 256
OUT_DIM = 64
HEADS = 4
D = HEADS * OUT_DIM  # 256
SLOPE = 0.01

NCORES = 8
NPC = N // NCORES          # 6250 real nodes per core
NPAD = 6272                # padded nodes per core (49 * 128)
NG = NPAD // 128           # 49 groups per core
NTAB = 50176               # padded table rows (392 * 128)
ROWE = 384                 # table row elems (bf16) = 768 bytes
LOW_LIM = 32768            # call-0 window [0, 32768)
HI_OFF = 17408             # call-1 window [17408, 50176)

_CACHE = {}


# ---------------------------------------------------------------- host prep

def _pair_nodes(lmin, lmax):
    """Pair 6272 nodes s.t. lmin[a]+lmin[b] <= 16 <= lmax[a]+lmax[b].
    Returns [3136, 2] array of node indices."""
    n = len(lmin)
    order = np.argsort(-(lmin.astype(np.int64) * 32 + lmax))
    a = order[: n // 2]
    b = order[n // 2:][::-1].copy()
    # repair pass: greedy swaps if any pair infeasible
    for _ in range(4):
        bad = np.where((lmin[a] + lmin[b] > 16) | (lmax[a] + lmax[b] < 16))[0]
        if len(bad) == 0:
            break
        for i in bad:
            done = False
            for j in range(len(a)):
                if i == j:
                    continue
                # try swapping b[i] and b[j]
                ok1 = lmin[a[i]] + lmin[b[j]] <= 16 <= lmax[a[i]] + lmax[b[j]]
                ok2 = lmin[a[j]] + lmin[b[i]] <= 16 <= lmax[a[j]] + lmax[b[i]]
                if ok1 and ok2:
                    b[i], b[j] = b[j], b[i]
                    done = True
                    break
            if not done:
                raise RuntimeError("node pairing failed")
    bad = np.where((lmin[a] + lmin[b] > 16) | (lmax[a] + lmax[b] < 16))[0]
    assert len(bad) == 0, "pairing infeasible"
    return np.stack([a, b], 1)


def _prep_core(src_core):
    """src_core: [6250, 16] int64 source node ids for this core's dsts.

    Returns dict with per-core host tensors:
      idx   [2, NG, 128, 64] int16   packed gather indices (call 0/1)
      eq3   [NG, 128, 16, 32] bf16   one-hot col masks (nl' per slot)
      bit4  [NG, 128, 16, 4] bf16    B-side indicator per slot
      perm  [NPAD] int64             perm[row_in_out] = core-local node id
    """
    E = np.zeros((NPAD, DEG), np.int64)
    E[:NPC] = src_core
    E[NPC:, :8] = 0        # pad nodes: 8 low edges (src 0)
    E[NPC:, 8:] = LOW_LIM  # + 8 high edges (src 32768)

    is_low = E < HI_OFF
    is_high = E >= LOW_LIM
    lmin = is_low.sum(1)                 # must go to call-0
    lmax = DEG - is_high.sum(1)          # max possible in call-0

    pairs = _pair_nodes(lmin, lmax)      # [3136, 2]

    # per-pair boundary n0 = # of A-edges in tile 0
    A = pairs[:, 0]
    B = pairs[:, 1]
    n0 = np.maximum(lmin[A], DEG - lmax[B])
    hi = np.minimum(lmax[A], DEG - lmin[B])
    assert np.all(n0 <= hi)

    # per node: edges sorted low, both, high
    cls = np.where(is_low, 0, np.where(is_high, 2, 1))
    sort_ix = np.argsort(cls, axis=1, kind="stable")
    Esort = np.take_along_axis(E, sort_ix, axis=1)  # [NPAD, 16]

    # A's tile-0 edges = Esort[A][:n0], tile-1 = rest; B: t0 = first 16-n0
    idx = np.zeros((2, NG, 1024), np.int64)
    eqq = np.zeros((NG, 128, 16), np.int64)    # nl' per slot
    bit = np.zeros((NG, 128, 16), np.int64)    # B-side indicator
    perm = np.zeros(NPAD, np.int64)

    q_all = np.arange(3136)
    g_all = q_all // 64
    qg = q_all % 64               # pair index within group
    c_all = qg // 8               # chunk
    r_all = qg % 8                # partition range /16
    perm[g_all * 128 + 2 * qg] = A
    perm[g_all * 128 + 2 * qg + 1] = B

    # slot tables, vectorized over pairs
    nA0 = n0                       # A edges in tile 0
    nA1 = DEG - n0                 # A edges in tile 1
    s = np.arange(DEG)[None, :]
    # tile 0: A's sorted edges [0:nA0], then B's sorted edges [0:16-nA0]
    # tile 1: A's sorted edges [nA0:16], then B's sorted edges [16-nA0:16]
    EA = Esort[A]  # [3136, 16]
    EB = Esort[B]
    t0 = np.where(s < nA0[:, None],
                  np.take_along_axis(EA, np.clip(s, 0, 15), 1),
                  np.take_along_axis(EB, np.clip(s - nA0[:, None], 0, 15), 1))
    t1 = np.where(s < nA1[:, None],
                  np.take_along_axis(EA, np.clip(s + nA0[:, None], 0, 15), 1),
                  np.take_along_axis(
                      EB, np.clip(s - nA1[:, None] + (DEG - nA0[:, None]), 0, 15), 1))
    assert t0.max() < LOW_LIM, "t0 window violation"
    assert t1.min() >= HI_OFF, "t1 window violation"

    # scatter into per-group slot layout: call t slot j: p = j%128, c = j//128
    # lane: chunk c = qg//8, partitions [16r, 16r+16), slot-in-lane s = p%16
    gg = np.broadcast_to(g_all[:, None], (3136, DEG))
    pp = r_all[:, None] * 16 + s          # partition [3136, 16]
    cc = np.broadcast_to(c_all[:, None], (3136, DEG))
    for t, tedges, bound in ((0, t0, nA0), (1, t1, nA1)):
        j = cc * 128 + pp
        vals = tedges if t == 0 else tedges - HI_OFF
        idx[t][gg, j] = vals
        bslot = (s >= bound[:, None]).astype(np.int64)
        bit[gg, pp, t * 8 + cc] = bslot
        nl = 2 * qg[:, None] + bslot
        eqq[gg, pp, t * 8 + cc] = nl % 32

    # expand eq -> one-hot, replicated per head: [NG, 128, 16, 4, 32]
    eye = np.arange(32)
    eq1 = (eqq[:, :, :, None] == eye[None, None, None, :])
    eq4 = np.broadcast_to(eq1[:, :, :, None, :], (NG, 128, 16, HEADS, 32))
    eq4 = eq4.astype(ml_dtypes.bfloat16)
    bit4 = np.broadcast_to(bit[:, :, :, None], (NG, 128, 16, HEADS)).astype(np.uint8)

    # pack idx wrapped: flat[j] at [j%16, j//16], replicated to 128 partitions
    idxp = np.zeros((2, NG, 128, 64), np.int16)
    for t in range(2):
        w = idx[t].reshape(NG, 64, 16).transpose(0, 2, 1).astype(np.int16)  # [NG,16,64]
        idxp[t] = np.tile(w, (1, 8, 1))

    return dict(idx=idxp, eq4=np.ascontiguousarray(eq4),
                bit4=np.ascontiguousarray(bit4), perm=perm)


def _host_prep(h, W, a_src, a_dst, src, dst):
    bf16 = ml_dtypes.bfloat16
    exp_dst = np.repeat(np.arange(N, dtype=np.int64), DEG)
    assert np.array_equal(dst.astype(np.int64), exp_dst), \
        "kernel requires dst == repeat(arange(N), DEG)"

    # Waug [256, 264] = [W_cat | u_src | u_dst]
    Wcat = W.astype(np.float64).transpose(1, 0, 2).reshape(IN_DIM, D)
    u_src = np.einsum("hid,hd->ih", W.astype(np.float64), a_src.astype(np.float64))
    u_dst = np.einsum("hid,hd->ih", W.astype(np.float64), a_dst.astype(np.float64))
    Waug = np.concatenate([Wcat, u_src, u_dst], 1).astype(np.float32)  # [256, 264]

    # h transposed, padded, tiled: ht3[t, k] = h[128t:128t+128, 128k:+128].T
    ht = np.zeros((IN_DIM, NTAB), np.float32)
    ht[:, :N] = h.T
    ht3 = ht.reshape(2, 128, NTAB // 128, 128).transpose(2, 1, 0, 3)  # [392,128,2,128]
    ht3 = np.ascontiguousarray(ht3).astype(bf16)

    waug_c = np.ascontiguousarray(Waug.reshape(2, 128, 264)).astype(bf16)
    udst_c = np.ascontiguousarray(u_dst.reshape(2, 128, HEADS)).astype(np.float32).astype(bf16)

    # consts
    maskab = np.zeros((128, 2, 32), np.float32)
    nl = np.arange(128)
    for side in range(2):
        for c in range(8):
            for hh in range(HEADS):
                maskab[:, side, c * 4 + hh] = ((nl % 2 == side) & (nl // 16 == c))
    patp = np.zeros((128, 128), np.float32)
    for nn in range(128):
        patp[nn, :] = ((nn // 2) % 8 == (np.arange(128) // 16))
    patp = patp.astype(bf16)
    ones1 = np.ones((128, 1), bf16)

    per_core = []
    src2 = src.astype(np.int64).reshape(N, DEG)
    for c in range(NCORES):
        pc = _prep_core(src2[c * NPC:(c + 1) * NPC])
        # htp: permuted h columns for ed computation
        pnodes = np.where(pc["perm"] < NPC, pc["perm"] + c * NPC, 0)
        htp = ht[:, pnodes]  # [256, 6272]
        htp3 = htp.reshape(2, 128, NG, 128).transpose(2, 1, 0, 3)
        pc["htp"] = np.ascontiguousarray(htp3).astype(bf16)
        per_core.append(pc)

    shared = dict(ht3=ht3, waug=waug_c, udst=udst_c,
                  maskab=maskab, patp=patp, ones1=ones1)
    return shared, per_core


# ---------------------------------------------------------------- device

def _build_nc(debug=False):
    import concourse.bacc as bacc
    import concourse.bass as bass
    import concourse.mybir as mybir
    import concourse.tile as tile

    dt = mybir.dt
    AF = mybir.ActivationFunctionType
    ALU = mybir.AluOpType

    nc = bacc.Bacc("TRN2", num_swdge_queues=4)

    ht3 = nc.dram_tensor("ht3", [NTAB // 128, 128, 2, 128], dt.bfloat16, kind="ExternalInput")
    htp = nc.dram_tensor("htp", [NG, 128, 2, 128], dt.bfloat16, kind="ExternalInput")
    waug = nc.dram_tensor("waug", [2, 128, 264], dt.bfloat16, kind="ExternalInput")
    udst = nc.dram_tensor("udst", [2, 128, HEADS], dt.bfloat16, kind="ExternalInput")
    idx = nc.dram_tensor("idx", [2, NG, 128, 64], dt.int16, kind="ExternalInput")
    eq4 = nc.dram_tensor("eq4", [NG, 128, 16, HEADS, 32], dt.bfloat16, kind="ExternalInput")
    bit4 = nc.dram_tensor("bit4", [NG, 128, 16, HEADS], dt.uint8, kind="ExternalInput")
    maskab = nc.dram_tensor("maskab", [128, 2, 32], dt.float32, kind="ExternalInput")
    patp = nc.dram_tensor("patp", [128, 128], dt.bfloat16, kind="ExternalInput")
    ones1 = nc.dram_tensor("ones1", [128, 1], dt.bfloat16, kind="ExternalInput")
    outp = nc.dram_tensor("outp", [NPAD, D], dt.float32, kind="ExternalOutput")
    ztab = nc.dram_tensor("ztab", [NTAB, ROWE], dt.bfloat16, kind="Internal")
    if debug:
        dbg_zt = nc.dram_tensor("dbg_zt", [NTAB, 264], dt.bfloat16, kind="ExternalOutput")
        dbg_g = nc.dram_tensor("dbg_g", [128, 16, ROWE], dt.bfloat16, kind="ExternalOutput")
        dbg_ex = nc.dram_tensor("dbg_ex", [128, 16, HEADS], dt.float32, kind="ExternalOutput")
        dbg_lhx = nc.dram_tensor("dbg_lhx", [128, 16, 128], dt.bfloat16, kind="ExternalOutput")
        dbg_den = nc.dram_tensor("dbg_den", [128, 4], dt.float32, kind="ExternalOutput")
        dbg_ed = nc.dram_tensor("dbg_ed", [128, NG, HEADS], dt.float32, kind="ExternalOutput")

    with tile.TileContext(nc) as tc:
        with (
            tc.tile_pool(name="const", bufs=1) as constp,
            tc.tile_pool(name="h_in", bufs=3) as hp,
            tc.tile_pool(name="zrow", bufs=3) as zp,
            tc.tile_pool(name="edall", bufs=1) as edallp,
            tc.tile_pool(name="gath", bufs=3) as gp,
            tc.tile_pool(name="ix", bufs=4) as ixp,
            tc.tile_pool(name="gdata", bufs=3) as gdp,
            tc.tile_pool(name="work", bufs=3) as wp,
            tc.tile_pool(name="lhx", bufs=2) as lhxp,
            tc.tile_pool(name="psum", bufs=2, space="PSUM") as psp,
            tc.tile_pool(name="outsb", bufs=3) as outsbp,
        ):
            # ---- constants to SBUF
            waug_sb = constp.tile([128, 2, 264], dt.bfloat16)
            nc.sync.dma_start(out=waug_sb[:], in_=waug.ap().rearrange("k p c -> p k c"))
            udst_sb = constp.tile([128, 2, HEADS], dt.bfloat16)
            nc.sync.dma_start(out=udst_sb[:], in_=udst.ap().rearrange("k p c -> p k c"))
            maskab_sb = constp.tile([128, 2, 32], dt.float32)
            nc.sync.dma_start(out=maskab_sb[:], in_=maskab[:, :, :])
            patp_sb = constp.tile([128, 128], dt.bfloat16)
            nc.sync.dma_start(out=patp_sb[:], in_=patp[:, :])
            ones_sb = constp.tile([128, 1], dt.bfloat16)
            nc.sync.dma_start(out=ones_sb[:], in_=ones1[:, :])

            # ---- Phase 1: z table
            for t in range(NTAB // 128):
                htile = hp.tile([128, 2, 128], dt.bfloat16)
                nc.sync.dma_start(out=htile[:], in_=ht3[t])
                ps = psp.tile([128, 264], dt.float32, tag="p1")
                nc.tensor.matmul(ps[:], lhsT=htile[:, 0, :], rhs=waug_sb[:, 0, :],
                                 start=True, stop=False)
                nc.tensor.matmul(ps[:], lhsT=htile[:, 1, :], rhs=waug_sb[:, 1, :],
                                 start=False, stop=True)
                zrow = zp.tile([128, 264], dt.bfloat16)
                if t % 2 == 0:
                    nc.vector.tensor_copy(out=zrow[:], in_=ps[:])
                else:
                    nc.scalar.activation(zrow[:], ps[:], AF.Identity)
                nc.sync.dma_start(out=ztab[t * 128:(t + 1) * 128, 0:264], in_=zrow[:])
                if debug:
                    nc.sync.dma_start(out=dbg_zt[t * 128:(t + 1) * 128, :], in_=zrow[:])

            # ---- Phase 1b: ed for own (permuted) nodes
            ed_all = edallp.tile([128, NG, HEADS], dt.float32)
            for g in range(NG):
                hptile = hp.tile([128, 2, 128], dt.bfloat16)
                nc.sync.dma_start(out=hptile[:], in_=htp[g])
                eps = psp.tile([128, HEADS], dt.float32, tag="den")
                nc.tensor.matmul(eps[:], lhsT=hptile[:, 0, :], rhs=udst_sb[:, 0, :],
                                 start=True, stop=False)
                nc.tensor.matmul(eps[:], lhsT=hptile[:, 1, :], rhs=udst_sb[:, 1, :],
                                 start=False, stop=True)
                nc.vector.tensor_copy(out=ed_all[:, g, :], in_=eps[:])
            if debug:
                nc.sync.dma_start(out=dbg_ed[:, :, :], in_=ed_all[:])

            # ---- Phase 2
            for g in range(NG):
                ix0 = ixp.tile([128, 64], dt.int16)
                nc.sync.dma_start(out=ix0[:], in_=idx[0, g])
                ix1 = ixp.tile([128, 64], dt.int16)
                nc.sync.dma_start(out=ix1[:], in_=idx[1, g])
                G = gp.tile([128, 16, ROWE], dt.bfloat16)
                nc.gpsimd.dma_gather(
                    out_ap=G[:, 0:8, :], in_ap=ztab[:, :], idxs_ap=ix0[:],
                    num_idxs=1024, num_idxs_reg=1024, elem_size=ROWE,
                    queue_num=(2 * g) % 4)
                nc.gpsimd.dma_gather(
                    out_ap=G[:, 8:16, :], in_ap=ztab[HI_OFF:, :], idxs_ap=ix1[:],
                    num_idxs=1024, num_idxs_reg=1024, elem_size=ROWE,
                    queue_num=(2 * g + 1) % 4)

                eqt = gdp.tile([128, 16, HEADS, 32], dt.bfloat16, tag="eqt")
                nc.sync.dma_start(out=eqt[:], in_=eq4[g])
                bitt = gdp.tile([128, 16, HEADS], dt.uint8, tag="bitt")
                nc.sync.dma_start(out=bitt[:], in_=bit4[g])

                # ed replicate+mask: edm[nl, s, (c,h)] = ed_g[nl, h] * maskab
                edm = wp.tile([128, 2, 8, HEADS], dt.bfloat16, tag="edm")
                edg_b = ed_all[:, g:g + 1, :].broadcast_to((128, 8, HEADS))
                for s_ in range(2):
                    nc.vector.tensor_tensor(
                        out=edm[:, s_, :, :], in0=edg_b,
                        in1=maskab_sb[:, s_, :].rearrange("p (c h) -> p c h", h=HEADS),
                        op=ALU.mult)
                edab_ps = psp.tile([128, 64], dt.float32, tag="edab")
                nc.tensor.matmul(edab_ps[:], lhsT=patp_sb[:],
                                 rhs=edm[:].rearrange("p s c h -> p (s c h)"),
                                 start=True, stop=True)
                edab = wp.tile([128, 2, 8, HEADS], dt.float32, tag="edab_sb")
                nc.scalar.activation(edab[:].rearrange("p s c h -> p (s c h)"),
                                     edab_ps[:], AF.Identity)

                # e_pre[p, (t,c), h] = es + edA + (edB-edA)*bit
                esf = wp.tile([128, 16, HEADS], dt.float32, tag="esf")
                nc.vector.tensor_copy(out=esf[:], in_=G[:, :, 256:256 + HEADS])
                epre = wp.tile([128, 2, 8, HEADS], dt.float32, tag="epre")
                for t_ in range(2):
                    nc.vector.select(
                        out=epre[:, t_, :, :],
                        mask=bitt[:, 8 * t_:8 * t_ + 8, :],
                        on_true=edab[:, 1, :, :],
                        on_false=edab[:, 0, :, :])
                nc.vector.tensor_add(
                    out=epre[:],
                    in0=epre[:],
                    in1=esf[:].rearrange("p (t c) h -> p t c h", t=2))
                # leaky relu + exp on ACT
                ex = wp.tile([128, 16, HEADS], dt.float32, tag="ex")
                nc.scalar.activation(ex[:], epre[:].rearrange("p t c h -> p (t c) h"),
                                     AF.Lrelu, alpha=SLOPE)
                exb = wp.tile([128, 16, HEADS], dt.bfloat16, tag="exb")
                nc.scalar.activation(exb[:], ex[:], AF.Exp)

                # LHx [128, 16, 128] bf16: col = 32h + q', val = exb * eq
                lhx = lhxp.tile([128, 16, 128], dt.bfloat16)
                nc.vector.tensor_tensor(
                    out=lhx[:].rearrange("p t (h q) -> p t h q", q=32),
                    in0=exb[:].to_broadcast([128, 16, HEADS, 32]),
                    in1=eqt[:],
                    op=ALU.mult)

                if debug and g == 0:
                    nc.sync.dma_start(out=dbg_g[:], in_=G[:])
                    nc.sync.dma_start(out=dbg_ex[:], in_=ex[:])
                    nc.sync.dma_start(out=dbg_lhx[:], in_=lhx[:])
                # weighted matmuls + denominators
                den_ps = psp.tile([128, 4], dt.float32, tag="den")
                rec = wp.tile([128, 4], dt.float32, tag="rec")
                osb = outsbp.tile([128, 4, 256], dt.float32)
                for w in range(4):
                    ow_ps = psp.tile([128, 256], dt.float32, tag="ow")
                    tcs = [(0, 2 * w), (0, 2 * w + 1), (1, 2 * w), (1, 2 * w + 1)]
                    for i, (t_, c_) in enumerate(tcs):
                        lh = lhx[:, t_ * 8 + c_, :]
                        nc.tensor.matmul(ow_ps[:], lhsT=lh,
                                         rhs=G[:, t_ * 8 + c_, 0:256],
                                         start=(i == 0), stop=(i == 3))
                        nc.tensor.matmul(den_ps[:, w:w + 1], lhsT=lh,
                                         rhs=ones_sb[:],
                                         start=(i == 0), stop=(i == 3))
                    nc.vector.reciprocal(out=rec[:, w:w + 1], in_=den_ps[:, w:w + 1])
                    if debug and g == 0 and w == 3:
                        nc.vector.tensor_copy(out=rec[:, :], in_=den_ps[:, :])
                        nc.sync.dma_start(out=dbg_den[:], in_=rec[:])
                        nc.vector.reciprocal(out=rec[:, :], in_=den_ps[:, :])
                    if w % 2 == 0:
                        nc.scalar.activation(osb[:, w, :], ow_ps[:], AF.Identity,
                                             scale=rec[:, w:w + 1])
                    else:
                        nc.vector.tensor_scalar_mul(osb[:, w, :], ow_ps[:],
                                                    rec[:, w:w + 1])
                # unscrambling output writes: psum col 32h+nl' -> row 32w+nl'
                for w in range(4):
                    for hh in range(4):
                        nc.sync.dma_start(
                            out=outp[g * 128 + 32 * w:g * 128 + 32 * w + 32,
                                     64 * hh:64 * hh + 64],
                            in_=osb[32 * hh:32 * hh + 32, w, 64 * hh:64 * hh + 64])

    nc.compile()
    return nc


# ---------------------------------------------------------------- entry

def kernel(h, W, a_src, a_dst, src, dst):
    from concourse.bass_utils import run_bass_kernel_spmd

    key = "built"
    if key not in _CACHE:
        _CACHE[key] = _build_nc()
    nc = _CACHE[key]

    shared, per_core = _host_prep(h, W, a_src, a_dst, src, dst)
    in_maps = []
    for c in range(NCORES):
        pc = per_core[c]
        in_maps.append(dict(
            ht3=shared["ht3"], waug=shared["waug"], udst=shared["udst"],
            maskab=shared["maskab"], patp=shared["patp"], ones1=shared["ones1"],
            htp=pc["htp"], idx=pc["idx"], eq4=pc["eq4"], bit4=pc["bit4"],
        ))
    res = run_bass_kernel_spmd(nc, in_maps, core_ids=list(range(NCORES)))

    out = np.zeros((N, D), np.float32)
    for c in range(NCORES):
        op = res.results[c]["outp"]  # [NPAD, 256] in permuted order
        perm = per_core[c]["perm"]
        real = perm < NPC
        out[c * NPC + perm[real]] = op[real]
    return out


# revision 12
# speedup vs baseline: 1.6492x; 1.6492x over previous
"""Multi-head GAT layer (4 heads, cat) on 8 TRN2 NeuronCores.

Strategy (edge-parallel by dst, per-core full z table):
  - dst = repeat(arange(N), 16)  (verified): edges pre-grouped by dst node.
  - Core c owns dst nodes [6250c, 6250c+6250) and their 100k edges.
  - Phase 1 (per core): z_aug = h @ [W | u_src | u_dst] -> HBM table
    [50176 rows x 384 bf16] (row = 256 z + 4 es + 4 ed + pad to 768B).
  - Phase 2: per group of 128 dst nodes, gather the 2048 source rows with
    two dma_gather calls (int16 idx => two table windows [0,32768) and
    [17408,50176), host-paired "pair-lane" layout so both calls are exactly
    1024 indices with zero padding), then:
      logits e = es[src] + ed[dst] (ed via pattern matmuls), leaky-relu, exp
      weighted segment sums + denominators via PE matmuls with data-driven
      one-hot masks (host-precomputed EQ tensors), division folded into the
      PSUM evacuation scale.
  - Output rows are in host-chosen (paired) order; host inverts at the end.
"""

import numpy as np
import ml_dtypes

N = 50000
DEG = 16
IN_DIM = 256
OUT_DIM = 64
HEADS = 4
D = HEADS * OUT_DIM  # 256
SLOPE = 0.01

NCORES = 8
NPC = N // NCORES          # 6250 real nodes per core
NPAD = 6272                # padded nodes per core (49 * 128)
NG = NPAD // 128           # 49 groups per core
NTAB = 50176               # padded table rows (392 * 128)
ROWE = 384                 # table row elems (bf16) = 768 bytes
NSUP = 98                  # P1 super-tiles (4 node-tiles each)
LOW_LIM = 32768            # call-0 window [0, 32768)
HI_OFF = 17408             # call-1 window [17408, 50176)

_CACHE = {}


# ---------------------------------------------------------------- host prep

def _pair_nodes(lmin, lmax):
    """Pair 6272 nodes s.t. lmin[a]+lmin[b] <= 16 <= lmax[a]+lmax[b].
    Returns [3136, 2] array of node indices."""
    n = len(lmin)
    order = np.argsort(-(lmin.astype(np.int64) * 32 + lmax))
    a = order[: n // 2]
    b = order[n // 2:][::-1].copy()
    # repair pass: greedy swaps if any pair infeasible
    for _ in range(4):
        bad = np.where((lmin[a] + lmin[b] > 16) | (lmax[a] + lmax[b] < 16))[0]
        if len(bad) == 0:
            break
        for i in bad:
            done = False
            for j in range(len(a)):
                if i == j:
                    continue
                # try swapping b[i] and b[j]
                ok1 = lmin[a[i]] + lmin[b[j]] <= 16 <= lmax[a[i]] + lmax[b[j]]
                ok2 = lmin[a[j]] + lmin[b[i]] <= 16 <= lmax[a[j]] + lmax[b[i]]
                if ok1 and ok2:
                    b[i], b[j] = b[j], b[i]
                    done = True
                    break
            if not done:
                raise RuntimeError("node pairing failed")
    bad = np.where((lmin[a] + lmin[b] > 16) | (lmax[a] + lmax[b] < 16))[0]
    assert len(bad) == 0, "pairing infeasible"
    return np.stack([a, b], 1)


def _prep_core(src_core):
    """src_core: [6250, 16] int64 source node ids for this core's dsts.

    Returns dict with per-core host tensors:
      idx   [2, NG, 128, 64] int16   packed gather indices (call 0/1)
      eq3   [NG, 128, 16, 32] bf16   one-hot col masks (nl' per slot)
      bit4  [NG, 128, 16, 4] bf16    B-side indicator per slot
      perm  [NPAD] int64             perm[row_in_out] = core-local node id
    """
    E = np.zeros((NPAD, DEG), np.int64)
    E[:NPC] = src_core
    E[NPC:, :8] = 0        # pad nodes: 8 low edges (src 0)
    E[NPC:, 8:] = LOW_LIM  # + 8 high edges (src 32768)

    is_low = E < HI_OFF
    is_high = E >= LOW_LIM
    lmin = is_low.sum(1)                 # must go to call-0
    lmax = DEG - is_high.sum(1)          # max possible in call-0

    pairs = _pair_nodes(lmin, lmax)      # [3136, 2]

    # per-pair boundary n0 = # of A-edges in tile 0
    A = pairs[:, 0]
    B = pairs[:, 1]
    n0 = np.maximum(lmin[A], DEG - lmax[B])
    hi = np.minimum(lmax[A], DEG - lmin[B])
    assert np.all(n0 <= hi)

    # per node: edges sorted low, both, high
    cls = np.where(is_low, 0, np.where(is_high, 2, 1))
    sort_ix = np.argsort(cls, axis=1, kind="stable")
    Esort = np.take_along_axis(E, sort_ix, axis=1)  # [NPAD, 16]

    # A's tile-0 edges = Esort[A][:n0], tile-1 = rest; B: t0 = first 16-n0
    idx = np.zeros((2, NG, 1024), np.int64)
    eqq = np.zeros((NG, 128, 16), np.int64)    # nl' per slot
    bit = np.zeros((NG, 128, 16), np.int64)    # B-side indicator
    perm = np.zeros(NPAD, np.int64)

    q_all = np.arange(3136)
    g_all = q_all // 64
    qg = q_all % 64               # pair index within group
    c_all = qg // 8               # chunk
    r_all = qg % 8                # partition range /16
    perm[g_all * 128 + 2 * qg] = A
    perm[g_all * 128 + 2 * qg + 1] = B

    # slot tables, vectorized over pairs
    nA0 = n0                       # A edges in tile 0
    nA1 = DEG - n0                 # A edges in tile 1
    s = np.arange(DEG)[None, :]
    # tile 0: A's sorted edges [0:nA0], then B's sorted edges [0:16-nA0]
    # tile 1: A's sorted edges [nA0:16], then B's sorted edges [16-nA0:16]
    EA = Esort[A]  # [3136, 16]
    EB = Esort[B]
    t0 = np.where(s < nA0[:, None],
                  np.take_along_axis(EA, np.clip(s, 0, 15), 1),
                  np.take_along_axis(EB, np.clip(s - nA0[:, None], 0, 15), 1))
    t1 = np.where(s < nA1[:, None],
                  np.take_along_axis(EA, np.clip(s + nA0[:, None], 0, 15), 1),
                  np.take_along_axis(
                      EB, np.clip(s - nA1[:, None] + (DEG - nA0[:, None]), 0, 15), 1))
    assert t0.max() < LOW_LIM, "t0 window violation"
    assert t1.min() >= HI_OFF, "t1 window violation"

    # scatter into per-group slot layout: call t slot j: p = j%128, c = j//128
    # lane: chunk c = qg//8, partitions [16r, 16r+16), slot-in-lane s = p%16
    gg = np.broadcast_to(g_all[:, None], (3136, DEG))
    pp = r_all[:, None] * 16 + s          # partition [3136, 16]
    cc = np.broadcast_to(c_all[:, None], (3136, DEG))
    for t, tedges, bound in ((0, t0, nA0), (1, t1, nA1)):
        j = cc * 128 + pp
        vals = tedges if t == 0 else tedges - HI_OFF
        idx[t][gg, j] = vals
        bslot = (s >= bound[:, None]).astype(np.int64)
        bit[gg, pp, t * 8 + cc] = bslot
        nl = 2 * qg[:, None] + bslot
        eqq[gg, pp, t * 8 + cc] = nl % 32

    # expand eq -> one-hot, replicated per head: [NG, 128, 16, 4, 32]
    eye = np.arange(32)
    eq1 = (eqq[:, :, :, None] == eye[None, None, None, :])
    eq4 = np.broadcast_to(eq1[:, :, :, None, :], (NG, 128, 16, HEADS, 32))
    eq4 = eq4.astype(ml_dtypes.bfloat16)
    bit4 = np.broadcast_to(bit[:, :, :, None], (NG, 128, 16, HEADS)).astype(np.uint8)

    # pack idx wrapped: flat[j] at [j%16, j//16], replicated to 128 partitions
    idxp = np.zeros((2, NG, 128, 64), np.int16)
    for t in range(2):
        w = idx[t].reshape(NG, 64, 16).transpose(0, 2, 1).astype(np.int16)  # [NG,16,64]
        idxp[t] = np.tile(w, (1, 8, 1))

    return dict(idx=idxp, eq4=np.ascontiguousarray(eq4),
                bit4=np.ascontiguousarray(bit4), perm=perm)


def _host_prep(h, W, a_src, a_dst, src, dst):
    bf16 = ml_dtypes.bfloat16
    exp_dst = np.repeat(np.arange(N, dtype=np.int64), DEG)
    assert np.array_equal(dst.astype(np.int64), exp_dst), \
        "kernel requires dst == repeat(arange(N), DEG)"

    # Waug [256, 264] = [W_cat | u_src | u_dst]
    Wcat = W.astype(np.float64).transpose(1, 0, 2).reshape(IN_DIM, D)
    u_src = np.einsum("hid,hd->ih", W.astype(np.float64), a_src.astype(np.float64))
    u_dst = np.einsum("hid,hd->ih", W.astype(np.float64), a_dst.astype(np.float64))
    Waug = np.concatenate([Wcat, u_src, u_dst], 1).astype(np.float32)  # [256, 264]

    # h transposed, padded, tiled: [98, 128, 4, 2, 128]
    ht = np.zeros((IN_DIM, NTAB), np.float32)
    ht[:, :N] = h.T
    ht3 = ht.reshape(2, 128, NSUP, 4, 128).transpose(2, 1, 3, 0, 4)
    ht3 = np.ascontiguousarray(ht3).astype(bf16)  # [98, 128, 4, 2, 128]

    # waug with a zero column at 256: [z 0:256 | 0 | es 257:261 | ed 261:265]
    Waug265 = np.zeros((IN_DIM, 265), np.float32)
    Waug265[:, 0:256] = Waug[:, 0:256]
    Waug265[:, 257:265] = Waug[:, 256:264]
    waug_c = np.ascontiguousarray(Waug265.reshape(2, 128, 265)).astype(bf16)
    udst_c = np.ascontiguousarray(u_dst.reshape(2, 128, HEADS)).astype(np.float32).astype(bf16)

    # consts
    maskab = np.zeros((128, 2, 32), np.float32)
    nl = np.arange(128)
    for side in range(2):
        for c in range(8):
            for hh in range(HEADS):
                maskab[:, side, c * 4 + hh] = ((nl % 2 == side) & (nl // 16 == c))
    patp = np.zeros((128, 128), np.float32)
    for nn in range(128):
        patp[nn, :] = ((nn // 2) % 8 == (np.arange(128) // 16))
    patp = patp.astype(bf16)

    per_core = []
    src2 = src.astype(np.int64).reshape(N, DEG)
    for c in range(NCORES):
        pc = _prep_core(src2[c * NPC:(c + 1) * NPC])
        # htp: permuted h columns for ed computation
        pnodes = np.where(pc["perm"] < NPC, pc["perm"] + c * NPC, 0)
        htp = ht[:, pnodes]  # [256, 6272]
        htp3 = htp.reshape(2, 128, NG, 128).transpose(2, 1, 0, 3)
        pc["htp"] = np.ascontiguousarray(htp3).astype(bf16)
        per_core.append(pc)

    shared = dict(ht3=ht3, waug=waug_c, udst=udst_c,
                  maskab=maskab, patp=patp)
    return shared, per_core


# ---------------------------------------------------------------- device

def _build_nc(debug=False):
    import concourse.bacc as bacc
    import concourse.bass as bass
    import concourse.mybir as mybir
    import concourse.tile as tile

    dt = mybir.dt
    AF = mybir.ActivationFunctionType
    ALU = mybir.AluOpType

    nc = bacc.Bacc("TRN2", num_swdge_queues=4)

    ht3 = nc.dram_tensor("ht3", [NSUP, 128, 4, 2, 128], dt.bfloat16, kind="ExternalInput")
    htp = nc.dram_tensor("htp", [NG, 128, 2, 128], dt.bfloat16, kind="ExternalInput")
    waug = nc.dram_tensor("waug", [2, 128, 265], dt.bfloat16, kind="ExternalInput")
    udst = nc.dram_tensor("udst", [2, 128, HEADS], dt.bfloat16, kind="ExternalInput")
    idx = nc.dram_tensor("idx", [2, NG, 128, 64], dt.int16, kind="ExternalInput")
    eq4 = nc.dram_tensor("eq4", [NG, 128, 16, HEADS, 32], dt.bfloat16, kind="ExternalInput")
    bit4 = nc.dram_tensor("bit4", [NG, 128, 16, HEADS], dt.uint8, kind="ExternalInput")
    maskab = nc.dram_tensor("maskab", [128, 2, 32], dt.float32, kind="ExternalInput")
    patp = nc.dram_tensor("patp", [128, 128], dt.bfloat16, kind="ExternalInput")
    outp = nc.dram_tensor("outp", [NPAD, D], dt.float32, kind="ExternalOutput")
    ztab = nc.dram_tensor("ztab", [NTAB, ROWE], dt.bfloat16, kind="Internal")
    if debug:
        dbg_zt = nc.dram_tensor("dbg_zt", [NTAB, 265], dt.bfloat16, kind="ExternalOutput")
        dbg_g = nc.dram_tensor("dbg_g", [128, 16, ROWE], dt.bfloat16, kind="ExternalOutput")
        dbg_ex = nc.dram_tensor("dbg_ex", [128, 16, HEADS], dt.float32, kind="ExternalOutput")
        dbg_lhx = nc.dram_tensor("dbg_lhx", [128, 16, 128], dt.bfloat16, kind="ExternalOutput")
        dbg_ed = nc.dram_tensor("dbg_ed", [128, NG, HEADS], dt.float32, kind="ExternalOutput")

    with tile.TileContext(nc) as tc:
        with (
            tc.tile_pool(name="const", bufs=1) as constp,
            tc.tile_pool(name="h_in", bufs=3) as hp,
            tc.tile_pool(name="zrow", bufs=3) as zp,
            tc.tile_pool(name="edall", bufs=1) as edallp,
            tc.tile_pool(name="gath", bufs=4) as gp,
            tc.tile_pool(name="gdata", bufs=4) as gdp,
            tc.tile_pool(name="work", bufs=3) as wp,
            tc.tile_pool(name="lhx", bufs=2) as lhxp,
            tc.tile_pool(name="psum", bufs=2, space="PSUM") as psp,
            tc.tile_pool(name="outsb", bufs=3) as outsbp,
        ):
            # ---- constants / resident data to SBUF
            waug_sb = constp.tile([128, 2, 265], dt.bfloat16)
            nc.sync.dma_start(out=waug_sb[:], in_=waug.ap().rearrange("k p c -> p k c"))
            udst_sb = constp.tile([128, 2, HEADS], dt.bfloat16)
            nc.sync.dma_start(out=udst_sb[:], in_=udst.ap().rearrange("k p c -> p k c"))
            maskab_sb = constp.tile([128, 2, 32], dt.float32)
            nc.sync.dma_start(out=maskab_sb[:], in_=maskab[:, :, :])
            patp_sb = constp.tile([128, 128], dt.bfloat16)
            nc.sync.dma_start(out=patp_sb[:], in_=patp[:, :])
            ixall = constp.tile([128, 2, NG, 64], dt.int16)
            nc.sync.dma_start(out=ixall[:], in_=idx.ap().rearrange("t g p w -> p (t g) w")
                              .rearrange("p (t g) w -> p t g w", t=2))
            bitall = constp.tile([128, NG, 16, HEADS], dt.uint8)
            nc.scalar.dma_start(out=bitall[:],
                                in_=bit4.ap().rearrange("g p t h -> p g (t h)")
                                .rearrange("p g (t h) -> p g t h", t=16))

            # ---- Phase 1: z table (4 node-tiles per super-tile)
            for t in range(NSUP):
                htile = hp.tile([128, 4, 2, 128], dt.bfloat16)
                nc.sync.dma_start(out=htile[:], in_=ht3[t])
                zrow = zp.tile([128, 4, 265], dt.bfloat16)
                for j in range(4):
                    ps = psp.tile([128, 265], dt.float32, tag="p1")
                    nc.tensor.matmul(ps[:], lhsT=htile[:, j, 0, :],
                                     rhs=waug_sb[:, 0, :], start=True, stop=False)
                    nc.tensor.matmul(ps[:], lhsT=htile[:, j, 1, :],
                                     rhs=waug_sb[:, 1, :], start=False, stop=True)
                    if j % 2 == 0:
                        nc.vector.tensor_copy(out=zrow[:, j, :], in_=ps[:])
                    else:
                        nc.scalar.activation(zrow[:, j, :], ps[:], AF.Identity)
                nc.vector.memset(zrow[:, :, 256:257], 1.0)
                nc.scalar.dma_start(
                    out=ztab[t * 512:(t + 1) * 512, 0:265]
                        .rearrange("(j p) c -> p j c", p=128),
                    in_=zrow[:])
                if debug:
                    nc.sync.dma_start(
                        out=dbg_zt[t * 512:(t + 1) * 512, :]
                            .rearrange("(j p) c -> p j c", p=128),
                        in_=zrow[:])

            # ---- Phase 1b: ed for own (permuted) nodes
            ed_all = edallp.tile([128, NG, HEADS], dt.float32)
            for g in range(NG):
                hptile = hp.tile([128, 2, 128], dt.bfloat16, tag="hptile")
                nc.scalar.dma_start(out=hptile[:], in_=htp[g])
                eps = psp.tile([128, HEADS], dt.float32, tag="den")
                nc.tensor.matmul(eps[:], lhsT=hptile[:, 0, :], rhs=udst_sb[:, 0, :],
                                 start=True, stop=False)
                nc.tensor.matmul(eps[:], lhsT=hptile[:, 1, :], rhs=udst_sb[:, 1, :],
                                 start=False, stop=True)
                nc.vector.tensor_copy(out=ed_all[:, g, :], in_=eps[:])
            if debug:
                nc.sync.dma_start(out=dbg_ed[:, :, :], in_=ed_all[:])

            # ---- Phase 2
            for g in range(NG):
                G = gp.tile([128, 16, ROWE], dt.bfloat16)
                nc.gpsimd.dma_gather(
                    out_ap=G[:, 0:8, :], in_ap=ztab[:, :], idxs_ap=ixall[:, 0, g, :],
                    num_idxs=1024, num_idxs_reg=1024, elem_size=ROWE,
                    queue_num=(2 * g) % 4)
                nc.gpsimd.dma_gather(
                    out_ap=G[:, 8:16, :], in_ap=ztab[HI_OFF:, :], idxs_ap=ixall[:, 1, g, :],
                    num_idxs=1024, num_idxs_reg=1024, elem_size=ROWE,
                    queue_num=(2 * g + 1) % 4)

                eqt = gdp.tile([128, 16, HEADS, 32], dt.bfloat16, tag="eqt")
                nc.scalar.dma_start(out=eqt[:], in_=eq4[g])

                # ed replicate+mask: edm[nl, s, (c,h)] = ed_g[nl, h] * maskab
                edm = wp.tile([128, 2, 8, HEADS], dt.bfloat16, tag="edm")
                edg_b = ed_all[:, g:g + 1, :].broadcast_to((128, 8, HEADS))
                for s_ in range(2):
                    nc.vector.tensor_tensor(
                        out=edm[:, s_, :, :], in0=edg_b,
                        in1=maskab_sb[:, s_, :].rearrange("p (c h) -> p c h", h=HEADS),
                        op=ALU.mult)
                edab_ps = psp.tile([128, 64], dt.float32, tag="edab")
                nc.tensor.matmul(edab_ps[:], lhsT=patp_sb[:],
                                 rhs=edm[:].rearrange("p s c h -> p (s c h)"),
                                 start=True, stop=True)
                edab = wp.tile([128, 2, 8, HEADS], dt.float32, tag="edab_sb")
                nc.scalar.activation(edab[:].rearrange("p s c h -> p (s c h)"),
                                     edab_ps[:], AF.Identity)

                # e_pre[p, (t,c), h] = es + edA + (edB-edA)*bit
                esf = wp.tile([128, 16, HEADS], dt.float32, tag="esf")
                nc.vector.tensor_copy(out=esf[:], in_=G[:, :, 257:257 + HEADS])
                epre = wp.tile([128, 2, 8, HEADS], dt.float32, tag="epre")
                for t_ in range(2):
                    nc.vector.select(
                        out=epre[:, t_, :, :],
                        mask=bitall[:, g, 8 * t_:8 * t_ + 8, :],
                        on_true=edab[:, 1, :, :],
                        on_false=edab[:, 0, :, :])
                nc.vector.tensor_add(
                    out=epre[:],
                    in0=epre[:],
                    in1=esf[:].rearrange("p (t c) h -> p t c h", t=2))
                # leaky relu on DVE: e = epre - 0.99*min(epre, 0)
                neg = wp.tile([128, 16, HEADS], dt.float32, tag="neg")
                nc.vector.tensor_scalar_min(
                    neg[:], epre[:].rearrange("p t c h -> p (t c) h"), 0.0)
                ex = wp.tile([128, 16, HEADS], dt.float32, tag="ex")
                nc.vector.scalar_tensor_tensor(
                    out=ex[:], in0=neg[:], scalar=-(1.0 - SLOPE), op0=ALU.mult,
                    in1=epre[:].rearrange("p t c h -> p (t c) h"), op1=ALU.add)
                exb = wp.tile([128, 16, HEADS], dt.bfloat16, tag="exb")
                nc.scalar.activation(exb[:], ex[:], AF.Exp)

                # LHx [128, 16, 128] bf16: col = 32h + q', val = exb * eq
                lhx = lhxp.tile([128, 16, 128], dt.bfloat16)
                nc.vector.tensor_tensor(
                    out=lhx[:].rearrange("p t (h q) -> p t h q", q=32),
                    in0=exb[:].to_broadcast([128, 16, HEADS, 32]),
                    in1=eqt[:],
                    op=ALU.mult)
                if debug and g == 0:
                    nc.sync.dma_start(out=dbg_g[:], in_=G[:])
                    nc.sync.dma_start(out=dbg_ex[:], in_=ex[:])
                    nc.sync.dma_start(out=dbg_lhx[:], in_=lhx[:])

                # weighted matmuls; denominator rides along as rhs column 256
                rec = wp.tile([128, 4], dt.float32, tag="rec")
                osb = outsbp.tile([128, 4, 256], dt.float32)
                for w in range(4):
                    ow_ps = psp.tile([128, 257], dt.float32, tag="ow")
                    tcs = [(0, 2 * w), (0, 2 * w + 1), (1, 2 * w), (1, 2 * w + 1)]
                    for i, (t_, c_) in enumerate(tcs):
                        nc.tensor.matmul(ow_ps[:], lhsT=lhx[:, t_ * 8 + c_, :],
                                         rhs=G[:, t_ * 8 + c_, 0:257],
                                         start=(i == 0), stop=(i == 3))
                    nc.vector.reciprocal(out=rec[:, w:w + 1], in_=ow_ps[:, 256:257])
                    nc.vector.tensor_scalar_mul(osb[:, w, :], ow_ps[:, 0:256],
                                                rec[:, w:w + 1])
                # output: one DMA per head-block, unscrambling cols/rows
                for hh in range(4):
                    nc.sync.dma_start(
                        out=outp[g * 128:(g + 1) * 128, 64 * hh:64 * hh + 64]
                            .rearrange("(w n) d -> n w d", w=4),
                        in_=osb[32 * hh:32 * hh + 32, :, 64 * hh:64 * hh + 64])

    nc.compile()
    return nc


# ---------------------------------------------------------------- entry

def kernel(h, W, a_src, a_dst, src, dst):
    from concourse.bass_utils import run_bass_kernel_spmd

    key = "built"
    if key not in _CACHE:
        _CACHE[key] = _build_nc()
    nc = _CACHE[key]

    shared, per_core = _host_prep(h, W, a_src, a_dst, src, dst)
    in_maps = []
    for c in range(NCORES):
        pc = per_core[c]
        in_maps.append(dict(
            ht3=shared["ht3"], waug=shared["waug"], udst=shared["udst"],
            maskab=shared["maskab"], patp=shared["patp"],
            htp=pc["htp"], idx=pc["idx"], eq4=pc["eq4"], bit4=pc["bit4"],
        ))
    res = run_bass_kernel_spmd(nc, in_maps, core_ids=list(range(NCORES)))

    out = np.zeros((N, D), np.float32)
    for c in range(NCORES):
        op = res.results[c]["outp"]  # [NPAD, 256] in permuted order
        perm = per_core[c]["perm"]
        real = perm < NPC
        out[c * NPC + perm[real]] = op[real]
    return out


# revision 13
# speedup vs baseline: 1.6557x; 1.0040x over previous
"""Multi-head GAT layer (4 heads, cat) on 8 TRN2 NeuronCores.

Strategy (edge-parallel by dst, per-core full z table):
  - dst = repeat(arange(N), 16)  (verified): edges pre-grouped by dst node.
  - Core c owns dst nodes [6250c, 6250c+6250) and their 100k edges.
  - Phase 1 (per core): z_aug = h @ [W | u_src | u_dst] -> HBM table
    [50176 rows x 384 bf16] (row = 256 z + 4 es + 4 ed + pad to 768B).
  - Phase 2: per group of 128 dst nodes, gather the 2048 source rows with
    two dma_gather calls (int16 idx => two table windows [0,32768) and
    [17408,50176), host-paired "pair-lane" layout so both calls are exactly
    1024 indices with zero padding), then:
      logits e = es[src] + ed[dst] (ed via pattern matmuls), leaky-relu, exp
      weighted segment sums + denominators via PE matmuls with data-driven
      one-hot masks (host-precomputed EQ tensors), division folded into the
      PSUM evacuation scale.
  - Output rows are in host-chosen (paired) order; host inverts at the end.
"""

import numpy as np
import ml_dtypes

N = 50000
DEG = 16
IN_DIM = 256
OUT_DIM = 64
HEADS = 4
D = HEADS * OUT_DIM  # 256
SLOPE = 0.01

NCORES = 8
NPC = N // NCORES          # 6250 real nodes per core
NPAD = 6272                # padded nodes per core (49 * 128)
NG = NPAD // 128           # 49 groups per core
NTAB = 50176               # padded table rows (392 * 128)
ROWE = 384                 # table row elems (bf16) = 768 bytes
NSUP = 98                  # P1 super-tiles (4 node-tiles each)
LOW_LIM = 32768            # call-0 window [0, 32768)
HI_OFF = 17408             # call-1 window [17408, 50176)

_CACHE = {}


# ---------------------------------------------------------------- host prep

def _pair_nodes(lmin, lmax):
    """Pair 6272 nodes s.t. lmin[a]+lmin[b] <= 16 <= lmax[a]+lmax[b].
    Returns [3136, 2] array of node indices."""
    n = len(lmin)
    order = np.argsort(-(lmin.astype(np.int64) * 32 + lmax))
    a = order[: n // 2]
    b = order[n // 2:][::-1].copy()
    # repair pass: greedy swaps if any pair infeasible
    for _ in range(4):
        bad = np.where((lmin[a] + lmin[b] > 16) | (lmax[a] + lmax[b] < 16))[0]
        if len(bad) == 0:
            break
        for i in bad:
            done = False
            for j in range(len(a)):
                if i == j:
                    continue
                # try swapping b[i] and b[j]
                ok1 = lmin[a[i]] + lmin[b[j]] <= 16 <= lmax[a[i]] + lmax[b[j]]
                ok2 = lmin[a[j]] + lmin[b[i]] <= 16 <= lmax[a[j]] + lmax[b[i]]
                if ok1 and ok2:
                    b[i], b[j] = b[j], b[i]
                    done = True
                    break
            if not done:
                raise RuntimeError("node pairing failed")
    bad = np.where((lmin[a] + lmin[b] > 16) | (lmax[a] + lmax[b] < 16))[0]
    assert len(bad) == 0, "pairing infeasible"
    return np.stack([a, b], 1)


def _prep_core(src_core):
    """src_core: [6250, 16] int64 source node ids for this core's dsts.

    Returns dict with per-core host tensors:
      idx   [2, NG, 128, 64] int16   packed gather indices (call 0/1)
      eq3   [NG, 128, 16, 32] bf16   one-hot col masks (nl' per slot)
      bit4  [NG, 128, 16, 4] bf16    B-side indicator per slot
      perm  [NPAD] int64             perm[row_in_out] = core-local node id
    """
    E = np.zeros((NPAD, DEG), np.int64)
    E[:NPC] = src_core
    E[NPC:, :8] = 0        # pad nodes: 8 low edges (src 0)
    E[NPC:, 8:] = LOW_LIM  # + 8 high edges (src 32768)

    is_low = E < HI_OFF
    is_high = E >= LOW_LIM
    lmin = is_low.sum(1)                 # must go to call-0
    lmax = DEG - is_high.sum(1)          # max possible in call-0

    pairs = _pair_nodes(lmin, lmax)      # [3136, 2]

    # per-pair boundary n0 = # of A-edges in tile 0
    A = pairs[:, 0]
    B = pairs[:, 1]
    n0 = np.maximum(lmin[A], DEG - lmax[B])
    hi = np.minimum(lmax[A], DEG - lmin[B])
    assert np.all(n0 <= hi)

    # per node: edges sorted low, both, high
    cls = np.where(is_low, 0, np.where(is_high, 2, 1))
    sort_ix = np.argsort(cls, axis=1, kind="stable")
    Esort = np.take_along_axis(E, sort_ix, axis=1)  # [NPAD, 16]

    # A's tile-0 edges = Esort[A][:n0], tile-1 = rest; B: t0 = first 16-n0
    idx = np.zeros((2, NG, 1024), np.int64)
    eqq = np.zeros((NG, 128, 16), np.int64)    # nl' per slot
    bit = np.zeros((NG, 128, 16), np.int64)    # B-side indicator
    perm = np.zeros(NPAD, np.int64)

    q_all = np.arange(3136)
    g_all = q_all // 64
    qg = q_all % 64               # pair index within group
    c_all = qg // 8               # chunk
    r_all = qg % 8                # partition range /16
    perm[g_all * 128 + 2 * qg] = A
    perm[g_all * 128 + 2 * qg + 1] = B

    # slot tables, vectorized over pairs
    nA0 = n0                       # A edges in tile 0
    nA1 = DEG - n0                 # A edges in tile 1
    s = np.arange(DEG)[None, :]
    # tile 0: A's sorted edges [0:nA0], then B's sorted edges [0:16-nA0]
    # tile 1: A's sorted edges [nA0:16], then B's sorted edges [16-nA0:16]
    EA = Esort[A]  # [3136, 16]
    EB = Esort[B]
    t0 = np.where(s < nA0[:, None],
                  np.take_along_axis(EA, np.clip(s, 0, 15), 1),
                  np.take_along_axis(EB, np.clip(s - nA0[:, None], 0, 15), 1))
    t1 = np.where(s < nA1[:, None],
                  np.take_along_axis(EA, np.clip(s + nA0[:, None], 0, 15), 1),
                  np.take_along_axis(
                      EB, np.clip(s - nA1[:, None] + (DEG - nA0[:, None]), 0, 15), 1))
    assert t0.max() < LOW_LIM, "t0 window violation"
    assert t1.min() >= HI_OFF, "t1 window violation"

    # scatter into per-group slot layout: call t slot j: p = j%128, c = j//128
    # lane: chunk c = qg//8, partitions [16r, 16r+16), slot-in-lane s = p%16
    gg = np.broadcast_to(g_all[:, None], (3136, DEG))
    pp = r_all[:, None] * 16 + s          # partition [3136, 16]
    cc = np.broadcast_to(c_all[:, None], (3136, DEG))
    for t, tedges, bound in ((0, t0, nA0), (1, t1, nA1)):
        j = cc * 128 + pp
        vals = tedges if t == 0 else tedges - HI_OFF
        idx[t][gg, j] = vals
        bslot = (s >= bound[:, None]).astype(np.int64)
        bit[gg, pp, t * 8 + cc] = bslot
        nl = 2 * qg[:, None] + bslot
        eqq[gg, pp, t * 8 + cc] = nl % 32

    # expand eq -> one-hot, replicated per head: [NG, 128, 16, 4, 32]
    eye = np.arange(32)
    eq1 = (eqq[:, :, :, None] == eye[None, None, None, :])
    eq4 = np.broadcast_to(eq1[:, :, :, None, :], (NG, 128, 16, HEADS, 32))
    eq4 = eq4.astype(ml_dtypes.bfloat16)
    bit4 = np.broadcast_to(bit[:, :, :, None], (NG, 128, 16, HEADS)).astype(np.uint8)

    # pack idx wrapped: flat[j] at [j%16, j//16], replicated to 128 partitions
    idxp = np.zeros((2, NG, 128, 64), np.int16)
    for t in range(2):
        w = idx[t].reshape(NG, 64, 16).transpose(0, 2, 1).astype(np.int16)  # [NG,16,64]
        idxp[t] = np.tile(w, (1, 8, 1))

    return dict(idx=idxp, eq4=np.ascontiguousarray(eq4),
                bit4=np.ascontiguousarray(bit4), perm=perm)


def _host_prep(h, W, a_src, a_dst, src, dst):
    bf16 = ml_dtypes.bfloat16
    exp_dst = np.repeat(np.arange(N, dtype=np.int64), DEG)
    assert np.array_equal(dst.astype(np.int64), exp_dst), \
        "kernel requires dst == repeat(arange(N), DEG)"

    # Waug [256, 264] = [W_cat | u_src | u_dst]
    Wcat = W.astype(np.float64).transpose(1, 0, 2).reshape(IN_DIM, D)
    u_src = np.einsum("hid,hd->ih", W.astype(np.float64), a_src.astype(np.float64))
    u_dst = np.einsum("hid,hd->ih", W.astype(np.float64), a_dst.astype(np.float64))
    Waug = np.concatenate([Wcat, u_src, u_dst], 1).astype(np.float32)  # [256, 264]

    # h transposed, padded, tiled: [98, 128, 4, 2, 128]
    ht = np.zeros((IN_DIM, NTAB), np.float32)
    ht[:, :N] = h.T
    ht3 = ht.reshape(2, 128, NSUP, 4, 128).transpose(2, 1, 3, 0, 4)
    ht3 = np.ascontiguousarray(ht3).astype(bf16)  # [98, 128, 4, 2, 128]

    # waug with a zero column at 256: [z 0:256 | 0 | es 257:261 | ed 261:265]
    Waug265 = np.zeros((IN_DIM, 265), np.float32)
    Waug265[:, 0:256] = Waug[:, 0:256]
    Waug265[:, 257:265] = Waug[:, 256:264]
    waug_c = np.ascontiguousarray(Waug265.reshape(2, 128, 265)).astype(bf16)
    udst_c = np.ascontiguousarray(u_dst.reshape(2, 128, HEADS)).astype(np.float32).astype(bf16)

    # consts
    maskab = np.zeros((128, 2, 32), np.float32)
    nl = np.arange(128)
    for side in range(2):
        for c in range(8):
            for hh in range(HEADS):
                maskab[:, side, c * 4 + hh] = ((nl % 2 == side) & (nl // 16 == c))
    patp = np.zeros((128, 128), np.float32)
    for nn in range(128):
        patp[nn, :] = ((nn // 2) % 8 == (np.arange(128) // 16))
    patp = patp.astype(bf16)

    per_core = []
    src2 = src.astype(np.int64).reshape(N, DEG)
    for c in range(NCORES):
        pc = _prep_core(src2[c * NPC:(c + 1) * NPC])
        # htp: permuted h columns for ed computation
        pnodes = np.where(pc["perm"] < NPC, pc["perm"] + c * NPC, 0)
        htp = ht[:, pnodes]  # [256, 6272]
        htp3 = htp.reshape(2, 128, NG, 128).transpose(2, 1, 0, 3)
        pc["htp"] = np.ascontiguousarray(htp3).astype(bf16)
        per_core.append(pc)

    shared = dict(ht3=ht3, waug=waug_c, udst=udst_c,
                  maskab=maskab, patp=patp)
    return shared, per_core


# ---------------------------------------------------------------- device

def _build_nc(debug=False):
    import concourse.bacc as bacc
    import concourse.bass as bass
    import concourse.mybir as mybir
    import concourse.tile as tile

    dt = mybir.dt
    AF = mybir.ActivationFunctionType
    ALU = mybir.AluOpType

    nc = bacc.Bacc("TRN2", num_swdge_queues=4)

    ht3 = nc.dram_tensor("ht3", [NSUP, 128, 4, 2, 128], dt.bfloat16, kind="ExternalInput")
    htp = nc.dram_tensor("htp", [NG, 128, 2, 128], dt.bfloat16, kind="ExternalInput")
    waug = nc.dram_tensor("waug", [2, 128, 265], dt.bfloat16, kind="ExternalInput")
    udst = nc.dram_tensor("udst", [2, 128, HEADS], dt.bfloat16, kind="ExternalInput")
    idx = nc.dram_tensor("idx", [2, NG, 128, 64], dt.int16, kind="ExternalInput")
    eq4 = nc.dram_tensor("eq4", [NG, 128, 16, HEADS, 32], dt.bfloat16, kind="ExternalInput")
    bit4 = nc.dram_tensor("bit4", [NG, 128, 16, HEADS], dt.uint8, kind="ExternalInput")
    maskab = nc.dram_tensor("maskab", [128, 2, 32], dt.float32, kind="ExternalInput")
    patp = nc.dram_tensor("patp", [128, 128], dt.bfloat16, kind="ExternalInput")
    outp = nc.dram_tensor("outp", [NPAD, D], dt.float32, kind="ExternalOutput")
    ztab = nc.dram_tensor("ztab", [NTAB, ROWE], dt.bfloat16, kind="Internal")
    if debug:
        dbg_zt = nc.dram_tensor("dbg_zt", [NTAB, 265], dt.bfloat16, kind="ExternalOutput")
        dbg_g = nc.dram_tensor("dbg_g", [128, 16, ROWE], dt.bfloat16, kind="ExternalOutput")
        dbg_ex = nc.dram_tensor("dbg_ex", [128, 16, HEADS], dt.float32, kind="ExternalOutput")
        dbg_lhx = nc.dram_tensor("dbg_lhx", [128, 16, 128], dt.bfloat16, kind="ExternalOutput")
        dbg_ed = nc.dram_tensor("dbg_ed", [128, NG, HEADS], dt.float32, kind="ExternalOutput")

    with tile.TileContext(nc) as tc:
        with (
            tc.tile_pool(name="const", bufs=1) as constp,
            tc.tile_pool(name="h_in", bufs=3) as hp,
            tc.tile_pool(name="zrow", bufs=3) as zp,
            tc.tile_pool(name="edall", bufs=1) as edallp,
            tc.tile_pool(name="gath", bufs=6) as gp,
            tc.tile_pool(name="gdata", bufs=4) as gdp,
            tc.tile_pool(name="work", bufs=3) as wp,
            tc.tile_pool(name="lhx", bufs=2) as lhxp,
            tc.tile_pool(name="psum", bufs=2, space="PSUM") as psp,
            tc.tile_pool(name="p1psum", bufs=4, space="PSUM") as p1ps,
            tc.tile_pool(name="outsb", bufs=3) as outsbp,
        ):
            # ---- constants / resident data to SBUF
            waug_sb = constp.tile([128, 2, 265], dt.bfloat16)
            nc.sync.dma_start(out=waug_sb[:], in_=waug.ap().rearrange("k p c -> p k c"))
            udst_sb = constp.tile([128, 2, HEADS], dt.bfloat16)
            nc.sync.dma_start(out=udst_sb[:], in_=udst.ap().rearrange("k p c -> p k c"))
            maskab_sb = constp.tile([128, 2, 32], dt.float32)
            nc.sync.dma_start(out=maskab_sb[:], in_=maskab[:, :, :])
            patp_sb = constp.tile([128, 128], dt.bfloat16)
            nc.sync.dma_start(out=patp_sb[:], in_=patp[:, :])
            ixall = constp.tile([128, 2, NG, 64], dt.int16)
            nc.sync.dma_start(out=ixall[:], in_=idx.ap().rearrange("t g p w -> p (t g) w")
                              .rearrange("p (t g) w -> p t g w", t=2))
            bitall = constp.tile([128, NG, 16, HEADS], dt.uint8)
            nc.scalar.dma_start(out=bitall[:],
                                in_=bit4.ap().rearrange("g p t h -> p g (t h)")
                                .rearrange("p g (t h) -> p g t h", t=16))

            # ---- Phase 1: z table (4 node-tiles per super-tile)
            for t in range(NSUP):
                htile = hp.tile([128, 4, 2, 128], dt.bfloat16)
                nc.sync.dma_start(out=htile[:], in_=ht3[t])
                zrow = zp.tile([128, 4, 265], dt.bfloat16)
                for j in range(4):
                    ps = p1ps.tile([128, 265], dt.float32, tag="p1")
                    nc.tensor.matmul(ps[:], lhsT=htile[:, j, 0, :],
                                     rhs=waug_sb[:, 0, :], start=True, stop=False)
                    nc.tensor.matmul(ps[:], lhsT=htile[:, j, 1, :],
                                     rhs=waug_sb[:, 1, :], start=False, stop=True)
                    nc.vector.tensor_copy(out=zrow[:, j, 0:136], in_=ps[:, 0:136])
                    nc.scalar.activation(zrow[:, j, 136:265], ps[:, 136:265],
                                         AF.Identity)
                nc.vector.memset(zrow[:, :, 256:257], 1.0)
                nc.scalar.dma_start(
                    out=ztab[t * 512:(t + 1) * 512, 0:265]
                        .rearrange("(j p) c -> p j c", p=128),
                    in_=zrow[:])
                if debug:
                    nc.sync.dma_start(
                        out=dbg_zt[t * 512:(t + 1) * 512, :]
                            .rearrange("(j p) c -> p j c", p=128),
                        in_=zrow[:])

            # ---- Phase 1b: ed for own (permuted) nodes
            ed_all = edallp.tile([128, NG, HEADS], dt.float32)
            for g in range(NG):
                hptile = hp.tile([128, 2, 128], dt.bfloat16, tag="hptile")
                nc.scalar.dma_start(out=hptile[:], in_=htp[g])
                eps = psp.tile([128, HEADS], dt.float32, tag="aux")
                nc.tensor.matmul(eps[:], lhsT=hptile[:, 0, :], rhs=udst_sb[:, 0, :],
                                 start=True, stop=False)
                nc.tensor.matmul(eps[:], lhsT=hptile[:, 1, :], rhs=udst_sb[:, 1, :],
                                 start=False, stop=True)
                nc.vector.tensor_copy(out=ed_all[:, g, :], in_=eps[:])
            if debug:
                nc.sync.dma_start(out=dbg_ed[:, :, :], in_=ed_all[:])

            # ---- Phase 2
            for g in range(NG):
                G = gp.tile([128, 16, ROWE], dt.bfloat16)
                nc.gpsimd.dma_gather(
                    out_ap=G[:, 0:8, :], in_ap=ztab[:, :], idxs_ap=ixall[:, 0, g, :],
                    num_idxs=1024, num_idxs_reg=1024, elem_size=ROWE,
                    queue_num=(2 * g) % 4)
                nc.gpsimd.dma_gather(
                    out_ap=G[:, 8:16, :], in_ap=ztab[HI_OFF:, :], idxs_ap=ixall[:, 1, g, :],
                    num_idxs=1024, num_idxs_reg=1024, elem_size=ROWE,
                    queue_num=(2 * g + 1) % 4)

                eqt = gdp.tile([128, 16, HEADS, 32], dt.bfloat16, tag="eqt")
                nc.scalar.dma_start(out=eqt[:], in_=eq4[g])

                # ed replicate+mask: edm[nl, s, (c,h)] = ed_g[nl, h] * maskab
                edm = wp.tile([128, 2, 8, HEADS], dt.bfloat16, tag="edm")
                edg_b = ed_all[:, g:g + 1, :].broadcast_to((128, 8, HEADS))
                for s_ in range(2):
                    nc.vector.tensor_tensor(
                        out=edm[:, s_, :, :], in0=edg_b,
                        in1=maskab_sb[:, s_, :].rearrange("p (c h) -> p c h", h=HEADS),
                        op=ALU.mult)
                edab_ps = psp.tile([128, 64], dt.float32, tag="aux")
                nc.tensor.matmul(edab_ps[:], lhsT=patp_sb[:],
                                 rhs=edm[:].rearrange("p s c h -> p (s c h)"),
                                 start=True, stop=True)
                edab = wp.tile([128, 2, 8, HEADS], dt.float32, tag="edab_sb")
                nc.scalar.activation(edab[:].rearrange("p s c h -> p (s c h)"),
                                     edab_ps[:], AF.Identity)

                # e_pre[p, (t,c), h] = es + edA + (edB-edA)*bit
                esf = wp.tile([128, 16, HEADS], dt.float32, tag="esf")
                nc.vector.tensor_copy(out=esf[:], in_=G[:, :, 257:257 + HEADS])
                epre = wp.tile([128, 2, 8, HEADS], dt.float32, tag="epre")
                for t_ in range(2):
                    nc.vector.select(
                        out=epre[:, t_, :, :],
                        mask=bitall[:, g, 8 * t_:8 * t_ + 8, :],
                        on_true=edab[:, 1, :, :],
                        on_false=edab[:, 0, :, :])
                nc.vector.tensor_add(
                    out=epre[:],
                    in0=epre[:],
                    in1=esf[:].rearrange("p (t c) h -> p t c h", t=2))
                # leaky relu on DVE: e = epre - 0.99*min(epre, 0)
                neg = wp.tile([128, 16, HEADS], dt.float32, tag="neg")
                nc.vector.tensor_scalar_min(
                    neg[:], epre[:].rearrange("p t c h -> p (t c) h"), 0.0)
                ex = wp.tile([128, 16, HEADS], dt.float32, tag="ex")
                nc.vector.scalar_tensor_tensor(
                    out=ex[:], in0=neg[:], scalar=-(1.0 - SLOPE), op0=ALU.mult,
                    in1=epre[:].rearrange("p t c h -> p (t c) h"), op1=ALU.add)
                exb = wp.tile([128, 16, HEADS], dt.bfloat16, tag="exb")
                nc.scalar.activation(exb[:], ex[:], AF.Exp)

                # LHx [128, 16, 128] bf16: col = 32h + q', val = exb * eq
                lhx = lhxp.tile([128, 16, 128], dt.bfloat16)
                nc.vector.tensor_tensor(
                    out=lhx[:].rearrange("p t (h q) -> p t h q", q=32),
                    in0=exb[:].to_broadcast([128, 16, HEADS, 32]),
                    in1=eqt[:],
                    op=ALU.mult)
                if debug and g == 0:
                    nc.sync.dma_start(out=dbg_g[:], in_=G[:])
                    nc.sync.dma_start(out=dbg_ex[:], in_=ex[:])
                    nc.sync.dma_start(out=dbg_lhx[:], in_=lhx[:])

                # weighted matmuls; denominator rides along as rhs column 256
                rec = wp.tile([128, 4], dt.float32, tag="rec")
                osb = outsbp.tile([128, 4, 256], dt.float32)
                for w in range(4):
                    ow_ps = psp.tile([128, 257], dt.float32, tag="ow")
                    tcs = [(0, 2 * w), (0, 2 * w + 1), (1, 2 * w), (1, 2 * w + 1)]
                    for i, (t_, c_) in enumerate(tcs):
                        nc.tensor.matmul(ow_ps[:], lhsT=lhx[:, t_ * 8 + c_, :],
                                         rhs=G[:, t_ * 8 + c_, 0:257],
                                         start=(i == 0), stop=(i == 3))
                    nc.vector.reciprocal(out=rec[:, w:w + 1], in_=ow_ps[:, 256:257])
                    nc.vector.tensor_scalar_mul(osb[:, w, :], ow_ps[:, 0:256],
                                                rec[:, w:w + 1])
                # output: one DMA per head-block, unscrambling cols/rows
                for hh in range(4):
                    nc.sync.dma_start(
                        out=outp[g * 128:(g + 1) * 128, 64 * hh:64 * hh + 64]
                            .rearrange("(w n) d -> n w d", w=4),
                        in_=osb[32 * hh:32 * hh + 32, :, 64 * hh:64 * hh + 64])

    nc.compile()
    return nc


# ---------------------------------------------------------------- entry

def kernel(h, W, a_src, a_dst, src, dst):
    from concourse.bass_utils import run_bass_kernel_spmd

    key = "built"
    if key not in _CACHE:
        _CACHE[key] = _build_nc()
    nc = _CACHE[key]

    shared, per_core = _host_prep(h, W, a_src, a_dst, src, dst)
    in_maps = []
    for c in range(NCORES):
        pc = per_core[c]
        in_maps.append(dict(
            ht3=shared["ht3"], waug=shared["waug"], udst=shared["udst"],
            maskab=shared["maskab"], patp=shared["patp"],
            htp=pc["htp"], idx=pc["idx"], eq4=pc["eq4"], bit4=pc["bit4"],
        ))
    res = run_bass_kernel_spmd(nc, in_maps, core_ids=list(range(NCORES)))

    out = np.zeros((N, D), np.float32)
    for c in range(NCORES):
        op = res.results[c]["outp"]  # [NPAD, 256] in permuted order
        perm = per_core[c]["perm"]
        real = perm < NPC
        out[c * NPC + perm[real]] = op[real]
    return out


# revision 14
# speedup vs baseline: 1.8515x; 1.1183x over previous
"""Multi-head GAT layer (4 heads, cat) on 8 TRN2 NeuronCores.

Strategy (edge-parallel by dst, per-core full z table):
  - dst = repeat(arange(N), 16)  (verified): edges pre-grouped by dst node.
  - Core c owns dst nodes [6250c, 6250c+6250) and their 100k edges.
  - Phase 1 (per core): z_aug = h @ [W | u_src | u_dst] -> HBM table
    [50176 rows x 384 bf16] (row = 256 z + 4 es + 4 ed + pad to 768B).
  - Phase 2: per group of 128 dst nodes, gather the 2048 source rows with
    two dma_gather calls (int16 idx => two table windows [0,32768) and
    [17408,50176), host-paired "pair-lane" layout so both calls are exactly
    1024 indices with zero padding), then:
      logits e = es[src] + ed[dst] (ed via pattern matmuls), leaky-relu, exp
      weighted segment sums + denominators via PE matmuls with data-driven
      one-hot masks (host-precomputed EQ tensors), division folded into the
      PSUM evacuation scale.
  - Output rows are in host-chosen (paired) order; host inverts at the end.
"""

import numpy as np
import ml_dtypes

N = 50000
DEG = 16
IN_DIM = 256
OUT_DIM = 64
HEADS = 4
D = HEADS * OUT_DIM  # 256
SLOPE = 0.01

NCORES = 8
NPC = N // NCORES          # 6250 real nodes per core
NPAD = 6272                # padded nodes per core (49 * 128)
NG = NPAD // 128           # 49 groups per core
NTAB = 50176               # padded table rows (392 * 128)
ROWE = 384                 # table row elems (bf16) = 768 bytes
NSUP = 98                  # P1 super-tiles (4 node-tiles each)
LOW_LIM = 32768            # call-0 window [0, 32768)
HI_OFF = 17408             # call-1 window [17408, 50176)

_CACHE = {}


# ---------------------------------------------------------------- host prep

def _pair_nodes(lmin, lmax):
    """Pair 6272 nodes s.t. lmin[a]+lmin[b] <= 16 <= lmax[a]+lmax[b].
    Returns [3136, 2] array of node indices."""
    n = len(lmin)
    order = np.argsort(-(lmin.astype(np.int64) * 32 + lmax))
    a = order[: n // 2]
    b = order[n // 2:][::-1].copy()
    # repair pass: greedy swaps if any pair infeasible
    for _ in range(4):
        bad = np.where((lmin[a] + lmin[b] > 16) | (lmax[a] + lmax[b] < 16))[0]
        if len(bad) == 0:
            break
        for i in bad:
            done = False
            for j in range(len(a)):
                if i == j:
                    continue
                # try swapping b[i] and b[j]
                ok1 = lmin[a[i]] + lmin[b[j]] <= 16 <= lmax[a[i]] + lmax[b[j]]
                ok2 = lmin[a[j]] + lmin[b[i]] <= 16 <= lmax[a[j]] + lmax[b[i]]
                if ok1 and ok2:
                    b[i], b[j] = b[j], b[i]
                    done = True
                    break
            if not done:
                raise RuntimeError("node pairing failed")
    bad = np.where((lmin[a] + lmin[b] > 16) | (lmax[a] + lmax[b] < 16))[0]
    assert len(bad) == 0, "pairing infeasible"
    return np.stack([a, b], 1)


def _prep_core(src_core):
    """src_core: [6250, 16] int64 source node ids for this core's dsts.

    Returns dict with per-core host tensors:
      idx   [2, NG, 128, 64] int16   packed gather indices (call 0/1)
      eq3   [NG, 128, 16, 32] bf16   one-hot col masks (nl' per slot)
      bit4  [NG, 128, 16, 4] bf16    B-side indicator per slot
      perm  [NPAD] int64             perm[row_in_out] = core-local node id
    """
    E = np.zeros((NPAD, DEG), np.int64)
    E[:NPC] = src_core
    E[NPC:, :8] = 0        # pad nodes: 8 low edges (src 0)
    E[NPC:, 8:] = LOW_LIM  # + 8 high edges (src 32768)

    is_low = E < HI_OFF
    is_high = E >= LOW_LIM
    lmin = is_low.sum(1)                 # must go to call-0
    lmax = DEG - is_high.sum(1)          # max possible in call-0

    pairs = _pair_nodes(lmin, lmax)      # [3136, 2]

    # per-pair boundary n0 = # of A-edges in tile 0
    A = pairs[:, 0]
    B = pairs[:, 1]
    n0 = np.maximum(lmin[A], DEG - lmax[B])
    hi = np.minimum(lmax[A], DEG - lmin[B])
    assert np.all(n0 <= hi)

    # per node: edges sorted low, both, high
    cls = np.where(is_low, 0, np.where(is_high, 2, 1))
    sort_ix = np.argsort(cls, axis=1, kind="stable")
    Esort = np.take_along_axis(E, sort_ix, axis=1)  # [NPAD, 16]

    # A's tile-0 edges = Esort[A][:n0], tile-1 = rest; B: t0 = first 16-n0
    idx = np.zeros((2, NG, 1024), np.int64)
    eqq = np.zeros((NG, 128, 16), np.int64)    # nl' per slot
    bit = np.zeros((NG, 128, 16), np.int64)    # B-side indicator
    perm = np.zeros(NPAD, np.int64)

    q_all = np.arange(3136)
    g_all = q_all // 64
    qg = q_all % 64               # pair index within group
    c_all = qg // 8               # chunk
    r_all = qg % 8                # partition range /16
    perm[g_all * 128 + 2 * qg] = A
    perm[g_all * 128 + 2 * qg + 1] = B

    # slot tables, vectorized over pairs
    nA0 = n0                       # A edges in tile 0
    nA1 = DEG - n0                 # A edges in tile 1
    s = np.arange(DEG)[None, :]
    # tile 0: A's sorted edges [0:nA0], then B's sorted edges [0:16-nA0]
    # tile 1: A's sorted edges [nA0:16], then B's sorted edges [16-nA0:16]
    EA = Esort[A]  # [3136, 16]
    EB = Esort[B]
    t0 = np.where(s < nA0[:, None],
                  np.take_along_axis(EA, np.clip(s, 0, 15), 1),
                  np.take_along_axis(EB, np.clip(s - nA0[:, None], 0, 15), 1))
    t1 = np.where(s < nA1[:, None],
                  np.take_along_axis(EA, np.clip(s + nA0[:, None], 0, 15), 1),
                  np.take_along_axis(
                      EB, np.clip(s - nA1[:, None] + (DEG - nA0[:, None]), 0, 15), 1))
    assert t0.max() < LOW_LIM, "t0 window violation"
    assert t1.min() >= HI_OFF, "t1 window violation"

    # scatter into per-group slot layout: call t slot j: p = j%128, c = j//128
    # lane: chunk c = qg//8, partitions [16r, 16r+16), slot-in-lane s = p%16
    gg = np.broadcast_to(g_all[:, None], (3136, DEG))
    pp = r_all[:, None] * 16 + s          # partition [3136, 16]
    cc = np.broadcast_to(c_all[:, None], (3136, DEG))
    for t, tedges, bound in ((0, t0, nA0), (1, t1, nA1)):
        j = cc * 128 + pp
        vals = tedges if t == 0 else tedges - HI_OFF
        idx[t][gg, j] = vals
        bslot = (s >= bound[:, None]).astype(np.int64)
        bit[gg, pp, t * 8 + cc] = bslot
        nl = 2 * qg[:, None] + bslot
        eqq[gg, pp, t * 8 + cc] = nl % 32

    # expand eq -> one-hot: [NG, 128, 16, 32]
    eye = np.arange(32)
    eq4 = (eqq[:, :, :, None] == eye[None, None, None, :]).astype(ml_dtypes.bfloat16)
    bit4 = np.broadcast_to(bit[:, :, :, None], (NG, 128, 16, HEADS)).astype(np.uint8)

    # pack idx wrapped: flat[j] at [j%16, j//16], replicated to 128 partitions
    idxp = np.zeros((2, NG, 128, 64), np.int16)
    for t in range(2):
        w = idx[t].reshape(NG, 64, 16).transpose(0, 2, 1).astype(np.int16)  # [NG,16,64]
        idxp[t] = np.tile(w, (1, 8, 1))

    return dict(idx=idxp, eq4=np.ascontiguousarray(eq4),
                bit4=np.ascontiguousarray(bit4), perm=perm)


def _host_prep(h, W, a_src, a_dst, src, dst):
    bf16 = ml_dtypes.bfloat16
    exp_dst = np.repeat(np.arange(N, dtype=np.int64), DEG)
    assert np.array_equal(dst.astype(np.int64), exp_dst), \
        "kernel requires dst == repeat(arange(N), DEG)"

    # Waug [256, 264] = [W_cat | u_src | u_dst]
    Wcat = W.astype(np.float64).transpose(1, 0, 2).reshape(IN_DIM, D)
    u_src = np.einsum("hid,hd->ih", W.astype(np.float64), a_src.astype(np.float64))
    u_dst = np.einsum("hid,hd->ih", W.astype(np.float64), a_dst.astype(np.float64))
    Waug = np.concatenate([Wcat, u_src, u_dst], 1).astype(np.float32)  # [256, 264]

    # h transposed, padded, tiled: [98, 128, 4, 2, 128]
    ht = np.zeros((IN_DIM, NTAB), np.float32)
    ht[:, :N] = h.T
    ht3 = ht.reshape(2, 128, NSUP, 4, 128).transpose(2, 1, 3, 0, 4)
    ht3 = np.ascontiguousarray(ht3).astype(bf16)  # [98, 128, 4, 2, 128]

    # waug with a zero column at 256: [z 0:256 | 0 | es 257:261 | ed 261:265]
    Waug265 = np.zeros((IN_DIM, 265), np.float32)
    Waug265[:, 0:256] = Waug[:, 0:256]
    Waug265[:, 257:265] = Waug[:, 256:264]
    waug_c = np.ascontiguousarray(Waug265.reshape(2, 128, 265)).astype(bf16)
    udst_c = np.ascontiguousarray(u_dst.reshape(2, 128, HEADS)).astype(np.float32).astype(bf16)

    # consts
    maskab = np.zeros((128, 2, 32), np.float32)
    nl = np.arange(128)
    for side in range(2):
        for c in range(8):
            for hh in range(HEADS):
                maskab[:, side, c * 4 + hh] = ((nl % 2 == side) & (nl // 16 == c))
    patp = np.zeros((128, 128), np.float32)
    for nn in range(128):
        patp[nn, :] = ((nn // 2) % 8 == (np.arange(128) // 16))
    patp = patp.astype(bf16)

    per_core = []
    src2 = src.astype(np.int64).reshape(N, DEG)
    for c in range(NCORES):
        pc = _prep_core(src2[c * NPC:(c + 1) * NPC])
        # htp: permuted h columns for ed computation
        pnodes = np.where(pc["perm"] < NPC, pc["perm"] + c * NPC, 0)
        htp = ht[:, pnodes]  # [256, 6272]
        htp3 = htp.reshape(2, 128, NG, 128).transpose(2, 1, 0, 3)
        pc["htp"] = np.ascontiguousarray(htp3).astype(bf16)
        per_core.append(pc)

    shared = dict(ht3=ht3, waug=waug_c, udst=udst_c,
                  maskab=maskab, patp=patp)
    return shared, per_core


# ---------------------------------------------------------------- device

def _build_nc(debug=False):
    import concourse.bacc as bacc
    import concourse.bass as bass
    import concourse.mybir as mybir
    import concourse.tile as tile

    dt = mybir.dt
    AF = mybir.ActivationFunctionType
    ALU = mybir.AluOpType

    nc = bacc.Bacc("TRN2", num_swdge_queues=4, dynamic_dma_scratch_size=49152)

    ht3 = nc.dram_tensor("ht3", [NSUP, 128, 4, 2, 128], dt.bfloat16, kind="ExternalInput")
    htp = nc.dram_tensor("htp", [NG, 128, 2, 128], dt.bfloat16, kind="ExternalInput")
    waug = nc.dram_tensor("waug", [2, 128, 265], dt.bfloat16, kind="ExternalInput")
    udst = nc.dram_tensor("udst", [2, 128, HEADS], dt.bfloat16, kind="ExternalInput")
    idx = nc.dram_tensor("idx", [2, NG, 128, 64], dt.int16, kind="ExternalInput")
    eq4 = nc.dram_tensor("eq4", [NG, 128, 16, 32], dt.bfloat16, kind="ExternalInput")
    bit4 = nc.dram_tensor("bit4", [NG, 128, 16, HEADS], dt.uint8, kind="ExternalInput")
    maskab = nc.dram_tensor("maskab", [128, 2, 32], dt.float32, kind="ExternalInput")
    patp = nc.dram_tensor("patp", [128, 128], dt.bfloat16, kind="ExternalInput")
    outp = nc.dram_tensor("outp", [NPAD, D], dt.float32, kind="ExternalOutput")
    ztab = nc.dram_tensor("ztab", [NTAB, ROWE], dt.bfloat16, kind="Internal")
    if debug:
        dbg_zt = nc.dram_tensor("dbg_zt", [NTAB, 265], dt.bfloat16, kind="ExternalOutput")
        dbg_g = nc.dram_tensor("dbg_g", [128, 16, ROWE], dt.bfloat16, kind="ExternalOutput")
        dbg_ex = nc.dram_tensor("dbg_ex", [128, 16, HEADS], dt.float32, kind="ExternalOutput")
        dbg_lhx = nc.dram_tensor("dbg_lhx", [128, 16, 128], dt.bfloat16, kind="ExternalOutput")
        dbg_ed = nc.dram_tensor("dbg_ed", [128, NG, HEADS], dt.float32, kind="ExternalOutput")

    with tile.TileContext(nc) as tc:
        with (
            tc.tile_pool(name="const", bufs=1) as constp,
            tc.tile_pool(name="h_in", bufs=3) as hp,
            tc.tile_pool(name="zrow", bufs=3) as zp,
            tc.tile_pool(name="edall", bufs=1) as edallp,
            tc.tile_pool(name="gath", bufs=6) as gp,
            tc.tile_pool(name="gdata", bufs=4) as gdp,
            tc.tile_pool(name="work", bufs=3) as wp,
            tc.tile_pool(name="lhx", bufs=2) as lhxp,
            tc.tile_pool(name="psum", bufs=2, space="PSUM") as psp,
            tc.tile_pool(name="p1psum", bufs=4, space="PSUM") as p1ps,
            tc.tile_pool(name="outsb", bufs=3) as outsbp,
        ):
            # ---- constants / resident data to SBUF
            waug_sb = constp.tile([128, 2, 265], dt.bfloat16)
            nc.sync.dma_start(out=waug_sb[:], in_=waug.ap().rearrange("k p c -> p k c"))
            udst_sb = constp.tile([128, 2, HEADS], dt.bfloat16)
            nc.sync.dma_start(out=udst_sb[:], in_=udst.ap().rearrange("k p c -> p k c"))
            maskab_sb = constp.tile([128, 2, 32], dt.float32)
            nc.sync.dma_start(out=maskab_sb[:], in_=maskab[:, :, :])
            patp_sb = constp.tile([128, 128], dt.bfloat16)
            nc.sync.dma_start(out=patp_sb[:], in_=patp[:, :])
            ixall = constp.tile([128, 2, NG, 64], dt.int16)
            nc.sync.dma_start(out=ixall[:], in_=idx.ap().rearrange("t g p w -> p (t g) w")
                              .rearrange("p (t g) w -> p t g w", t=2))
            bitall = constp.tile([128, NG, 16, HEADS], dt.uint8)
            nc.scalar.dma_start(out=bitall[:],
                                in_=bit4.ap().rearrange("g p t h -> p g (t h)")
                                .rearrange("p g (t h) -> p g t h", t=16))

            # ---- Phase 1: z table (4 node-tiles per super-tile)
            for t in range(NSUP):
                htile = hp.tile([128, 4, 2, 128], dt.bfloat16)
                nc.sync.dma_start(out=htile[:], in_=ht3[t])
                zrow = zp.tile([128, 4, 265], dt.bfloat16)
                for j in range(4):
                    ps = p1ps.tile([128, 265], dt.float32, tag="p1")
                    nc.tensor.matmul(ps[:], lhsT=htile[:, j, 0, :],
                                     rhs=waug_sb[:, 0, :], start=True, stop=False)
                    nc.tensor.matmul(ps[:], lhsT=htile[:, j, 1, :],
                                     rhs=waug_sb[:, 1, :], start=False, stop=True)
                    nc.vector.tensor_copy(out=zrow[:, j, 0:136], in_=ps[:, 0:136])
                    nc.scalar.activation(zrow[:, j, 136:265], ps[:, 136:265],
                                         AF.Identity)
                nc.vector.memset(zrow[:, :, 256:257], 1.0)
                nc.scalar.dma_start(
                    out=ztab[t * 512:(t + 1) * 512, 0:265]
                        .rearrange("(j p) c -> p j c", p=128),
                    in_=zrow[:])
                if debug:
                    nc.sync.dma_start(
                        out=dbg_zt[t * 512:(t + 1) * 512, :]
                            .rearrange("(j p) c -> p j c", p=128),
                        in_=zrow[:])

            # ---- Phase 1b: ed for own (permuted) nodes
            ed_all = edallp.tile([128, NG, HEADS], dt.float32)
            for g in range(NG):
                hptile = hp.tile([128, 2, 128], dt.bfloat16, tag="hptile")
                nc.scalar.dma_start(out=hptile[:], in_=htp[g])
                eps = psp.tile([128, HEADS], dt.float32, tag="aux")
                nc.tensor.matmul(eps[:], lhsT=hptile[:, 0, :], rhs=udst_sb[:, 0, :],
                                 start=True, stop=False)
                nc.tensor.matmul(eps[:], lhsT=hptile[:, 1, :], rhs=udst_sb[:, 1, :],
                                 start=False, stop=True)
                nc.vector.tensor_copy(out=ed_all[:, g, :], in_=eps[:])
            if debug:
                nc.sync.dma_start(out=dbg_ed[:, :, :], in_=ed_all[:])

            # ---- Phase 2
            for g in range(NG):
                G = gp.tile([128, 16, ROWE], dt.bfloat16)
                nc.gpsimd.dma_gather(
                    out_ap=G[:, 0:8, :], in_ap=ztab[0:LOW_LIM, :], idxs_ap=ixall[:, 0, g, :],
                    num_idxs=1024, num_idxs_reg=1024, elem_size=ROWE,
                    queue_num=(2 * g) % 4)
                nc.gpsimd.dma_gather(
                    out_ap=G[:, 8:16, :], in_ap=ztab[HI_OFF:, :], idxs_ap=ixall[:, 1, g, :],
                    num_idxs=1024, num_idxs_reg=1024, elem_size=ROWE,
                    queue_num=(2 * g + 1) % 4)

                eqt = gdp.tile([128, 16, 32], dt.bfloat16, tag="eqt")
                nc.scalar.dma_start(out=eqt[:], in_=eq4[g])

                # ed replicate+mask: edm[nl, s, (c,h)] = ed_g[nl, h] * maskab
                edm = wp.tile([128, 2, 8, HEADS], dt.bfloat16, tag="edm")
                edg_b = ed_all[:, g:g + 1, :].broadcast_to((128, 8, HEADS))
                for s_ in range(2):
                    nc.vector.tensor_tensor(
                        out=edm[:, s_, :, :], in0=edg_b,
                        in1=maskab_sb[:, s_, :].rearrange("p (c h) -> p c h", h=HEADS),
                        op=ALU.mult)
                edab_ps = psp.tile([128, 64], dt.float32, tag="aux")
                nc.tensor.matmul(edab_ps[:], lhsT=patp_sb[:],
                                 rhs=edm[:].rearrange("p s c h -> p (s c h)"),
                                 start=True, stop=True)
                edab = wp.tile([128, 2, 8, HEADS], dt.float32, tag="edab_sb")
                nc.scalar.activation(edab[:].rearrange("p s c h -> p (s c h)"),
                                     edab_ps[:], AF.Identity)

                # e_pre[p, (t,c), h] = es + edA + (edB-edA)*bit
                esf = wp.tile([128, 16, HEADS], dt.float32, tag="esf")
                nc.vector.tensor_copy(out=esf[:], in_=G[:, :, 257:257 + HEADS])
                epre = wp.tile([128, 2, 8, HEADS], dt.float32, tag="epre")
                for t_ in range(2):
                    nc.vector.select(
                        out=epre[:, t_, :, :],
                        mask=bitall[:, g, 8 * t_:8 * t_ + 8, :],
                        on_true=edab[:, 1, :, :],
                        on_false=edab[:, 0, :, :])
                nc.vector.tensor_add(
                    out=epre[:],
                    in0=epre[:],
                    in1=esf[:].rearrange("p (t c) h -> p t c h", t=2))
                # leaky relu on DVE: e = epre - 0.99*min(epre, 0)
                neg = wp.tile([128, 16, HEADS], dt.float32, tag="neg")
                nc.vector.tensor_scalar_min(
                    neg[:], epre[:].rearrange("p t c h -> p (t c) h"), 0.0)
                ex = wp.tile([128, 16, HEADS], dt.float32, tag="ex")
                nc.vector.scalar_tensor_tensor(
                    out=ex[:], in0=neg[:], scalar=-(1.0 - SLOPE), op0=ALU.mult,
                    in1=epre[:].rearrange("p t c h -> p (t c) h"), op1=ALU.add)
                exb = wp.tile([128, 16, HEADS], dt.bfloat16, tag="exb")
                nc.scalar.activation(exb[:], ex[:], AF.Exp)

                # LHx [128, 16, 128] bf16: col = 32h + q', val = exb * eq
                lhx = lhxp.tile([128, 16, 128], dt.bfloat16)
                for h_ in range(HEADS):
                    nc.vector.tensor_tensor(
                        out=lhx[:, :, 32 * h_:32 * h_ + 32],
                        in0=exb[:, :, h_:h_ + 1].to_broadcast([128, 16, 32]),
                        in1=eqt[:],
                        op=ALU.mult)
                if debug and g == 0:
                    nc.sync.dma_start(out=dbg_g[:], in_=G[:])
                    nc.sync.dma_start(out=dbg_ex[:], in_=ex[:])
                    nc.sync.dma_start(out=dbg_lhx[:], in_=lhx[:])

                # weighted matmuls; denominator rides along as rhs column 256
                rec = wp.tile([128, 4], dt.float32, tag="rec")
                osb = outsbp.tile([128, 4, 256], dt.float32)
                for w in range(4):
                    ow_ps = psp.tile([128, 257], dt.float32, tag="ow")
                    tcs = [(0, 2 * w), (0, 2 * w + 1), (1, 2 * w), (1, 2 * w + 1)]
                    for i, (t_, c_) in enumerate(tcs):
                        nc.tensor.matmul(ow_ps[:], lhsT=lhx[:, t_ * 8 + c_, :],
                                         rhs=G[:, t_ * 8 + c_, 0:257],
                                         start=(i == 0), stop=(i == 3))
                    nc.vector.reciprocal(out=rec[:, w:w + 1], in_=ow_ps[:, 256:257])
                    nc.vector.tensor_scalar_mul(osb[:, w, :], ow_ps[:, 0:256],
                                                rec[:, w:w + 1])
                # output: one DMA per head-block, unscrambling cols/rows
                for hh in range(4):
                    nc.sync.dma_start(
                        out=outp[g * 128:(g + 1) * 128, 64 * hh:64 * hh + 64]
                            .rearrange("(w n) d -> n w d", w=4),
                        in_=osb[32 * hh:32 * hh + 32, :, 64 * hh:64 * hh + 64])

    nc.compile()
    return nc


# ---------------------------------------------------------------- entry

def kernel(h, W, a_src, a_dst, src, dst):
    from concourse.bass_utils import run_bass_kernel_spmd

    key = "built"
    if key not in _CACHE:
        _CACHE[key] = _build_nc()
    nc = _CACHE[key]

    shared, per_core = _host_prep(h, W, a_src, a_dst, src, dst)
    in_maps = []
    for c in range(NCORES):
        pc = per_core[c]
        in_maps.append(dict(
            ht3=shared["ht3"], waug=shared["waug"], udst=shared["udst"],
            maskab=shared["maskab"], patp=shared["patp"],
            htp=pc["htp"], idx=pc["idx"], eq4=pc["eq4"], bit4=pc["bit4"],
        ))
    res = run_bass_kernel_spmd(nc, in_maps, core_ids=list(range(NCORES)))

    out = np.zeros((N, D), np.float32)
    for c in range(NCORES):
        op = res.results[c]["outp"]  # [NPAD, 256] in permuted order
        perm = per_core[c]["perm"]
        real = perm < NPC
        out[c * NPC + perm[real]] = op[real]
    return out
